# revision 4
# baseline (speedup 1.0000x reference)
"""Trainium2 Bass kernel for nn_AttentionLayer (method='general' attention).

Reference computation:
    proj[l,b,:] = W @ enc[l,b,:] + bias          # [L,B,H]
    e[b,l]      = hidden[0,b,:] . proj[l,b,:]    # [B,L]
    attn        = softmax(e, axis=0 over b)[:, None, :]   # [B,1,L]

Algebraic rewrite (exact up to fp32 rounding):
    u[b,:] = hidden[0,b,:] @ W      (64x1024, tiny)
    c[b]   = hidden[0,b,:] . bias
    e[l,b] = u[b,:] . enc[l,b,:] + c[b]
which turns a 275-GFLOP matmul into a 512MB streaming dot-product problem.

Sharding: L axis (2048) split across 8 cores (256 rows each). The softmax is
over the batch axis, which stays fully local per core, so no collectives.

Per-core kernel:
  - PE computes u = h @ W (via on-chip transpose of h) and c = h @ bias.
  - The enc shard streams through SBUF as [128, 1024] tiles (rows = (l,b)
    pairs, cols = h); one fused DVE tensor_tensor_reduce per tile produces
    128 energies (the c bias folded in as the reduction's initial value).
  - Energies land in e_sb[128, 128] (e_sb[p, i] = E(pair 128*i + p)); one PE
    transpose puts each l's 64 batch energies contiguous along the free axis;
    segmented max/exp(+sum)/scale gives the softmax.
  - Output per core: [128, 128] where row c = [l=2c: b=0..63 | l=2c+1: b=0..63].
"""

import numpy as np

L_FULL, B, H = 2048, 64, 1024
N_CORES = 8
L_SHARD = L_FULL // N_CORES          # 256
PAIRS = L_SHARD * B                  # 16384 rows of the flattened shard
NT = PAIRS // 128                    # 128 column-tiles

_PROGRAM = None


def _build_program():
    import concourse.bacc as bacc
    import concourse.mybir as mybir
    from concourse import masks, tile

    f32 = mybir.dt.float32
    nc = bacc.Bacc(None)

    enc_in = nc.declare_dram_parameter("enc", [PAIRS, H], f32, isOutput=False)
    hid_in = nc.declare_dram_parameter("hid", [B, H], f32, isOutput=False)
    w_in = nc.declare_dram_parameter("w", [H, H], f32, isOutput=False)
    bv_in = nc.declare_dram_parameter("bv", [H], f32, isOutput=False)
    out_t = nc.declare_dram_parameter("attn", [128, 128], f32, isOutput=True)

    with tile.TileContext(nc) as tc:
        with (
            tc.tile_pool(name="const", bufs=1) as constp,
            tc.tile_pool(name="wpool", bufs=3) as wpool,
            tc.tile_pool(name="encp", bufs=12) as encp,
            tc.tile_pool(name="scr", bufs=2) as scrp,
            tc.tile_pool(name="small", bufs=4) as smallp,
            tc.tile_pool(name="psum", bufs=2, space="PSUM") as psump,
            tc.tile_pool(name="psum1", bufs=1, space="PSUM") as psump1,
        ):
            ident = constp.tile([128, 128], f32)
            masks.make_identity(nc, ident[:])

            # hidden [64,1024] -> hT [128, 8*64]: col block kb holds
            # hT[p, kb*64 + b] = hidden[b, kb*128 + p]
            hid_sb = constp.tile([B, H], f32)
            nc.sync.dma_start(hid_sb[:], hid_in[:])
            hT = constp.tile([128, 8 * B], f32)
            for kb in range(8):
                pt = psump.tile([128, B], f32)
                nc.tensor.transpose(
                    pt[:], hid_sb[:, kb * 128 : (kb + 1) * 128], ident[:B, :B]
                )
                nc.vector.tensor_copy(hT[:, kb * B : (kb + 1) * B], pt[:])

            # u = h @ W -> u2 [128, 1024] with the 64 rows duplicated on both
            # partition halves (pairs are ordered b-fastest, so partition p of
            # an enc tile needs u[p % 64, :]).
            u2 = constp.tile([128, H], f32)
            for nb in range(2):
                ups = psump.tile([B, 512], f32)
                for kb in range(8):
                    wt = wpool.tile([128, 512], f32)
                    nc.sync.dma_start(
                        wt[:], w_in[kb * 128 : (kb + 1) * 128, nb * 512 : (nb + 1) * 512]
                    )
                    nc.tensor.matmul(
                        ups[:],
                        hT[:, kb * B : (kb + 1) * B],
                        wt[:],
                        start=(kb == 0),
                        stop=(kb == 7),
                    )
                nc.vector.tensor_copy(u2[0:B, nb * 512 : (nb + 1) * 512], ups[:])
                nc.vector.tensor_copy(u2[B : 2 * B, nb * 512 : (nb + 1) * 512], ups[:])

            # c = h @ bias -> c2 [128, 1], duplicated across partition halves
            bv_sb = constp.tile([128, 8], f32)
            nc.sync.dma_start(bv_sb[:], bv_in.rearrange("(f p) -> p f", p=128))
            cps = psump.tile([B, 1], f32)
            for kb in range(8):
                nc.tensor.matmul(
                    cps[:],
                    hT[:, kb * B : (kb + 1) * B],
                    bv_sb[:, kb : kb + 1],
                    start=(kb == 0),
                    stop=(kb == 7),
                )
            c2 = constp.tile([128, 1], f32)
            nc.vector.tensor_copy(c2[0:B, :], cps[:])
            nc.vector.tensor_copy(c2[B : 2 * B, :], cps[:])

            # Main stream: 128 tiles of [128, 1024]; one fused DVE
            # multiply+reduce (scalar_tensor_tensor with accum_out) per tile.
            # (tensor_tensor_reduce crashes the exec unit on this HW/ucode.)
            e_sb = constp.tile([128, NT], f32)
            enc_v = enc_in.rearrange("(n p) h -> n p h", p=128)
            for i in range(NT):
                t = encp.tile([128, H], f32)
                nc.sync.dma_start(t[:], enc_v[i])
                scr = scrp.tile([128, H], f32)
                nc.vector.scalar_tensor_tensor(
                    out=scr[:],
                    in0=t[:],
                    scalar=1.0,
                    in1=u2[:],
                    op0=mybir.AluOpType.mult,
                    op1=mybir.AluOpType.mult,
                    accum_out=e_sb[:, i : i + 1],
                )

            # Add the bias term c[b] (per-partition since partitions are
            # b-fastest), then transpose.
            e_sb2 = smallp.tile([128, NT], f32)
            nc.vector.tensor_scalar_add(e_sb2[:], e_sb[:], c2[:, 0:1])

            # e_sb[p, i] = E(pair 128*i + p).  Transpose -> eT[c, p]: row c =
            # [l=2c: b 0..63 | l=2c+1: b 0..63]; softmax over each 64-col half.
            eT_ps = psump1.tile([128, 128], f32)
            nc.tensor.transpose(eT_ps[:], e_sb2[:], ident[:])
            eT = smallp.tile([128, 128], f32)
            nc.scalar.copy(eT[:], eT_ps[:])

            attn_sb = smallp.tile([128, 128], f32)
            for half in range(2):
                sl = slice(half * B, (half + 1) * B)
                nm = smallp.tile([128, 1], f32)
                nc.vector.tensor_reduce(
                    nm[:],
                    eT[:, sl],
                    axis=mybir.AxisListType.X,
                    op=mybir.AluOpType.max,
                    negate=True,
                )
                ex = smallp.tile([128, B], f32)
                ssum = smallp.tile([128, 1], f32)
                nc.scalar.activation(
                    ex[:],
                    eT[:, sl],
                    mybir.ActivationFunctionType.Exp,
                    bias=nm[:, 0:1],
                    scale=1.0,
                    accum_out=ssum[:],
                )
                rec = smallp.tile([128, 1], f32)
                nc.vector.reciprocal(rec[:], ssum[:])
                nc.vector.tensor_scalar_mul(attn_sb[:, sl], ex[:], rec[:, 0:1])

            nc.sync.dma_start(out_t[:], attn_sb[:])

    nc.finalize()
    return nc


def _get_program():
    global _PROGRAM
    if _PROGRAM is None:
        _PROGRAM = _build_program()
    return _PROGRAM


def kernel(**inputs) -> np.ndarray:
    from concourse.bass_utils import run_bass_kernel_spmd

    hidden = np.ascontiguousarray(np.asarray(inputs["hidden"], dtype=np.float32))
    enc = np.ascontiguousarray(np.asarray(inputs["encoder_outputs"], dtype=np.float32))
    W = np.ascontiguousarray(np.asarray(inputs["W"], dtype=np.float32))
    b = np.ascontiguousarray(np.asarray(inputs["b"], dtype=np.float32))

    nc = _get_program()
    h2 = hidden[0]
    in_maps = []
    for k in range(N_CORES):
        shard = np.ascontiguousarray(
            enc[k * L_SHARD : (k + 1) * L_SHARD].reshape(PAIRS, H)
        )
        in_maps.append({"enc": shard, "hid": h2, "w": W, "bv": b})

    res = run_bass_kernel_spmd(nc, in_maps, list(range(N_CORES)))

    outs = []
    for k in range(N_CORES):
        a = np.asarray(res.results[k]["attn"])  # [128, 128]
        # row c = [l=2c (64 b's) | l=2c+1 (64 b's)] -> [L_SHARD, B] -> [B, L_SHARD]
        outs.append(a.reshape(L_SHARD, B).T)
    out = np.concatenate(outs, axis=1)[:, None, :].astype(np.float32)
    return out


# revision 6
# speedup vs baseline: 1.0618x; 1.0618x over previous
"""Trainium2 Bass kernel for nn_AttentionLayer (method='general' attention).

Reference computation:
    proj[l,b,:] = W @ enc[l,b,:] + bias          # [L,B,H]
    e[b,l]      = hidden[0,b,:] . proj[l,b,:]    # [B,L]
    attn        = softmax(e, axis=0 over b)[:, None, :]   # [B,1,L]

Algebraic rewrite (exact up to fp32 rounding):
    u[b,:] = hidden[0,b,:] @ W      (64x1024, tiny)
    c[b]   = hidden[0,b,:] . bias
    e[l,b] = u[b,:] . enc[l,b,:] + c[b]
which turns a 275-GFLOP matmul into a 512MB streaming dot-product problem.

Sharding: L axis (2048) split across 8 cores (256 rows each). The softmax is
over the batch axis, which stays fully local per core, so no collectives.

Per-core kernel:
  - PE computes u = h @ W (via on-chip transpose of h) and c = h @ bias.
  - The enc shard streams through SBUF as [128, 1024] tiles (rows = (l,b)
    pairs, cols = h); one fused DVE tensor_tensor_reduce per tile produces
    128 energies (the c bias folded in as the reduction's initial value).
  - Energies land in e_sb[128, 128] (e_sb[p, i] = E(pair 128*i + p)); one PE
    transpose puts each l's 64 batch energies contiguous along the free axis;
    segmented max/exp(+sum)/scale gives the softmax.
  - Output per core: [128, 128] where row c = [l=2c: b=0..63 | l=2c+1: b=0..63].
"""

import numpy as np

L_FULL, B, H = 2048, 64, 1024
N_CORES = 8
L_SHARD = L_FULL // N_CORES          # 256
PAIRS = L_SHARD * B                  # 16384 rows of the flattened shard
NT = PAIRS // 128                    # 128 column-tiles

_PROGRAM = None


def _build_program():
    import concourse.bacc as bacc
    import concourse.mybir as mybir
    from concourse import masks, tile

    f32 = mybir.dt.float32
    nc = bacc.Bacc(None)

    enc_in = nc.declare_dram_parameter("enc", [PAIRS, H], f32, isOutput=False)
    hid_in = nc.declare_dram_parameter("hid", [B, H], f32, isOutput=False)
    w_in = nc.declare_dram_parameter("w", [H, H], f32, isOutput=False)
    bv_in = nc.declare_dram_parameter("bv", [H], f32, isOutput=False)
    out_t = nc.declare_dram_parameter("attn", [128, 128], f32, isOutput=True)

    with tile.TileContext(nc) as tc:
        with (
            tc.tile_pool(name="const", bufs=1) as constp,
            tc.tile_pool(name="wpool", bufs=3) as wpool,
            tc.tile_pool(name="encp", bufs=5) as encp,
            tc.tile_pool(name="scr", bufs=2) as scrp,
            tc.tile_pool(name="small", bufs=4) as smallp,
            tc.tile_pool(name="psum", bufs=2, space="PSUM") as psump,
            tc.tile_pool(name="psum1", bufs=1, space="PSUM") as psump1,
        ):
            ident = constp.tile([128, 128], f32)
            masks.make_identity(nc, ident[:])

            # hidden [64,1024] -> hT [128, 8*64]: col block kb holds
            # hT[p, kb*64 + b] = hidden[b, kb*128 + p]
            hid_sb = constp.tile([B, H], f32)
            nc.sync.dma_start(hid_sb[:], hid_in[:])
            hT = constp.tile([128, 8 * B], f32)
            for kb in range(8):
                pt = psump.tile([128, B], f32)
                nc.tensor.transpose(
                    pt[:], hid_sb[:, kb * 128 : (kb + 1) * 128], ident[:B, :B]
                )
                nc.vector.tensor_copy(hT[:, kb * B : (kb + 1) * B], pt[:])

            # u = h @ W -> u2 [128, 1024] with the 64 rows duplicated on both
            # partition halves (pairs are ordered b-fastest, so partition p of
            # an enc tile needs u[p % 64, :]).
            u2 = constp.tile([128, H], f32)
            for nb in range(2):
                ups = psump.tile([B, 512], f32)
                for kb in range(8):
                    wt = wpool.tile([128, 512], f32)
                    nc.sync.dma_start(
                        wt[:], w_in[kb * 128 : (kb + 1) * 128, nb * 512 : (nb + 1) * 512]
                    )
                    nc.tensor.matmul(
                        ups[:],
                        hT[:, kb * B : (kb + 1) * B],
                        wt[:],
                        start=(kb == 0),
                        stop=(kb == 7),
                    )
                nc.vector.tensor_copy(u2[0:B, nb * 512 : (nb + 1) * 512], ups[:])
                nc.vector.tensor_copy(u2[B : 2 * B, nb * 512 : (nb + 1) * 512], ups[:])

            # c = h @ bias -> c2 [128, 1], duplicated across partition halves
            bv_sb = constp.tile([128, 8], f32)
            nc.sync.dma_start(bv_sb[:], bv_in.rearrange("(f p) -> p f", p=128))
            cps = psump.tile([B, 1], f32)
            for kb in range(8):
                nc.tensor.matmul(
                    cps[:],
                    hT[:, kb * B : (kb + 1) * B],
                    bv_sb[:, kb : kb + 1],
                    start=(kb == 0),
                    stop=(kb == 7),
                )
            c2 = constp.tile([128, 1], f32)
            nc.vector.tensor_copy(c2[0:B, :], cps[:])
            nc.vector.tensor_copy(c2[B : 2 * B, :], cps[:])

            # Main stream: 128 tiles of [128, 1024]; one fused DVE
            # multiply+reduce (scalar_tensor_tensor with accum_out) per tile.
            # (tensor_tensor_reduce crashes the exec unit on this HW/ucode.)
            e_sb = constp.tile([128, NT], f32)
            S = 4  # tiles per DMA block: one 2MB dma_start per S column-tiles
            enc_v = enc_in.rearrange("(n s p) h -> n p s h", s=S, p=128)
            for i in range(NT // S):
                t = encp.tile([128, S, H], f32)
                nc.sync.dma_start(t[:], enc_v[i])
                for s in range(S):
                    scr = scrp.tile([128, H], f32)
                    nc.vector.scalar_tensor_tensor(
                        out=scr[:],
                        in0=t[:, s, :],
                        scalar=1.0,
                        in1=u2[:],
                        op0=mybir.AluOpType.mult,
                        op1=mybir.AluOpType.mult,
                        accum_out=e_sb[:, i * S + s : i * S + s + 1],
                    )

            # Add the bias term c[b] (per-partition since partitions are
            # b-fastest), then transpose.
            e_sb2 = smallp.tile([128, NT], f32)
            nc.vector.tensor_scalar_add(e_sb2[:], e_sb[:], c2[:, 0:1])

            # e_sb[p, i] = E(pair 128*i + p).  Transpose -> eT[c, p]: row c =
            # [l=2c: b 0..63 | l=2c+1: b 0..63]; softmax over each 64-col half.
            eT_ps = psump1.tile([128, 128], f32)
            nc.tensor.transpose(eT_ps[:], e_sb2[:], ident[:])
            eT = smallp.tile([128, 128], f32)
            nc.scalar.copy(eT[:], eT_ps[:])

            attn_sb = smallp.tile([128, 128], f32)
            for half in range(2):
                sl = slice(half * B, (half + 1) * B)
                nm = smallp.tile([128, 1], f32)
                nc.vector.tensor_reduce(
                    nm[:],
                    eT[:, sl],
                    axis=mybir.AxisListType.X,
                    op=mybir.AluOpType.max,
                    negate=True,
                )
                ex = smallp.tile([128, B], f32)
                ssum = smallp.tile([128, 1], f32)
                nc.scalar.activation(
                    ex[:],
                    eT[:, sl],
                    mybir.ActivationFunctionType.Exp,
                    bias=nm[:, 0:1],
                    scale=1.0,
                    accum_out=ssum[:],
                )
                rec = smallp.tile([128, 1], f32)
                nc.vector.reciprocal(rec[:], ssum[:])
                nc.vector.tensor_scalar_mul(attn_sb[:, sl], ex[:], rec[:, 0:1])

            nc.sync.dma_start(out_t[:], attn_sb[:])

    nc.finalize()
    return nc


def _get_program():
    global _PROGRAM
    if _PROGRAM is None:
        _PROGRAM = _build_program()
    return _PROGRAM


def kernel(**inputs) -> np.ndarray:
    from concourse.bass_utils import run_bass_kernel_spmd

    hidden = np.ascontiguousarray(np.asarray(inputs["hidden"], dtype=np.float32))
    enc = np.ascontiguousarray(np.asarray(inputs["encoder_outputs"], dtype=np.float32))
    W = np.ascontiguousarray(np.asarray(inputs["W"], dtype=np.float32))
    b = np.ascontiguousarray(np.asarray(inputs["b"], dtype=np.float32))

    nc = _get_program()
    h2 = hidden[0]
    in_maps = []
    for k in range(N_CORES):
        shard = np.ascontiguousarray(
            enc[k * L_SHARD : (k + 1) * L_SHARD].reshape(PAIRS, H)
        )
        in_maps.append({"enc": shard, "hid": h2, "w": W, "bv": b})

    res = run_bass_kernel_spmd(nc, in_maps, list(range(N_CORES)))

    outs = []
    for k in range(N_CORES):
        a = np.asarray(res.results[k]["attn"])  # [128, 128]
        # row c = [l=2c (64 b's) | l=2c+1 (64 b's)] -> [L_SHARD, B] -> [B, L_SHARD]
        outs.append(a.reshape(L_SHARD, B).T)
    out = np.concatenate(outs, axis=1)[:, None, :].astype(np.float32)
    return out
